# revision 16
# baseline (speedup 1.0000x reference)
"""Causal self-attention (B=4, T=2048, C=1024, H=16, D=64) on 8 NeuronCores.

Sharding: core c -> batch b = c//2 (data parallel), head-group hg = c%2
(tensor parallel: heads hg*8..hg*8+7; column-parallel qkv, row-parallel
proj). Each core computes a partial [T, C] projection output; the host
sums the two partials per batch (row-parallel all-reduce done host-side
during unshard) and adds b_proj.

All GEMMs run in float16 (1 cycle/row on the PE vs 2 passes for
fp32/fp32r; 11-bit mantissa keeps the end-to-end error ~1e-3).
Accumulation is always fp32 in PSUM.

Per-core device program (identical on all cores; only data differs):
  phase 1: qT,kT feature-major [128,4,T]; v token-major augmented with a
           ones column per head -> v_aug [T, 8*65]
  phase 2: per (q-tile qj of 512, head h): scores transposed
           sT[k_tok, q] = kT.T @ qT via K=64 matmuls; one Exp pass per
           [128,1024] PSUM group (softmax max-subtraction is skipped:
           scores are ~N(0,1) since 1/sqrt(D) is folded into w_q, so
           exp never overflows fp32 and softmax is shift-invariant;
           fp16 overflow in not-yet-masked upper-triangle entries is
           zeroed right after); causal masking via gpsimd affine_select
           on the diagonal blocks of p; AV accumulates yz[65, 512] with
           v_aug stationary -- row 64 is the softmax denominator Z;
           1/Z via ACT exp(-log(Z)); broadcast over partitions via a
           K=1 ones matmul; normalize on DVE; write yT feature-major.
  phase 3 (fused in the qj loop): out[512 tok slice, C] = yT.T @ wp.
"""

import numpy as np

import concourse.bass as bass
import concourse.mybir as mybir
from concourse.tile import TileContext
from concourse.bass_utils import run_bass_kernel_spmd

F32 = mybir.dt.float32
F16 = mybir.dt.float16

B, T, C = 4, 2048, 1024
H, D = 16, 64
HL = 8            # heads per core
CL = HL * D       # 512 local feature width

N_CORES = 8
NT = T // 512     # 4 q-tiles / token n-tiles of 512
MT = T // 128     # 16 token m-tiles
KF = C // 128     # 8 contraction tiles over C


def _split_waits(nc, cap=1):
    """walrus codegen limits sync-wait commands per ISA instruction.
    Hoist excess waits onto preceding same-engine nops."""
    n = 0
    for f in nc.m.functions:
        for blk in f.blocks:
            out = []
            for inst in blk.instructions:
                si = getattr(inst, "sync_info", None)
                if si is not None and si.on_wait and len(si.on_wait) > cap:
                    extra = list(si.on_wait[:-cap])
                    keep = list(si.on_wait[-cap:])
                    for w in extra:
                        nop = mybir.InstNoOp(name=f"wsplit-{n}", ins=[], outs=[])
                        nop.engine = inst.engine
                        nop.sync_info = mybir.SyncInfo(on_wait=[w], on_update=[])
                        out.append(nop)
                        n += 1
                    inst.sync_info = mybir.SyncInfo(
                        on_wait=keep, on_update=list(si.on_update)
                    )
                out.append(inst)
            blk.instructions = out
    return n


def build(split_for_hw=True):
    nc = bass.Bass()
    xT = nc.declare_dram_parameter("xT", [C, T], F16, isOutput=False)
    wk = nc.declare_dram_parameter("wk", [C, 3 * CL], F16, isOutput=False)
    bqk = nc.declare_dram_parameter("bqk", [128, 8], F32, isOutput=False)
    bv = nc.declare_dram_parameter("bv", [1, CL], F16, isOutput=False)
    wp = nc.declare_dram_parameter("wp", [CL, C], F16, isOutput=False)
    ones = nc.declare_dram_parameter("ones", [128, 128], F16, isOutput=False)
    selAB = nc.declare_dram_parameter("selAB", [128, 128], F16, isOutput=False)
    o = nc.declare_dram_parameter("o", [T, C], F32, isOutput=True)

    EXP = mybir.ActivationFunctionType.Exp
    LOG = mybir.ActivationFunctionType.Ln

    with TileContext(nc) as tc:
        with (
            tc.tile_pool(name="const", bufs=1) as cpool,
            tc.tile_pool(name="big", bufs=1) as big,
        ):
            # constants
            ones_sb = cpool.tile([128, 128], F16, tag="ones")
            bqk_sb = cpool.tile([128, 8], F32, tag="bqk")
            bv_sb = cpool.tile([1, CL], F16, tag="bv")
            selAB_sb = cpool.tile([128, 128], F16, tag="selAB")
            nc.sync.dma_start(out=selAB_sb[:], in_=selAB[:])
            nc.sync.dma_start(out=ones_sb[:], in_=ones[:])
            nc.sync.dma_start(out=bqk_sb[:], in_=bqk[:])
            nc.sync.dma_start(out=bv_sb[:], in_=bv[:])

            # persistent activations
            qT_sb = big.tile([128, NT, T], F16, tag="qT")    # 16 KB/part
            kT_sb = big.tile([128, NT, T], F16, tag="kT")    # 16 KB/part
            v_aug = [
                big.tile([128, HL * 65], F16, name=f"vaug{mt}", tag=f"vaug{mt}")
                for mt in range(MT)
            ]

            # ---------------- phase 1: qkv ----------------
            # mf-pair-outer order: feature block fb=u (q cols mf=u, k cols
            # mf=4+u) completes before pair u's attention needs it, so
            # phase 2 overlaps the back of phase 1. x stays fully resident
            # (fp16) as per-(nt,k) tiles for fine-grained DMA deps.
            with (
                tc.tile_pool(name="p1w", bufs=1) as p1w,
                tc.tile_pool(name="p1x", bufs=1) as p1x,
                tc.tile_pool(name="p1ps", bufs=5, space="PSUM") as p1ps,
                tc.tile_pool(name="p1vps", bufs=2, space="PSUM") as p1vps,
            ):
                wk_t = [
                    p1w.tile([128, 3 * CL], F16, name=f"wk{k}", tag=f"wk{k}")
                    for k in range(KF)
                ]
                wk_r = wk.rearrange("(ko ki) f -> ki ko f", ki=128)
                for k in range(KF):
                    nc.sync.dma_start(out=wk_t[k][:], in_=wk_r[:, k, :])
                x_t = [
                    [
                        p1x.tile([128, 512], F16, name=f"x{nt}_{k}", tag=f"x{nt}_{k}")
                        for k in range(KF)
                    ]
                    for nt in range(NT)
                ]
                xT_r = xT.rearrange("(ko ki) t -> ki ko t", ki=128)
                for nt in range(NT):
                    for k in range(KF):
                        nc.sync.dma_start(
                            out=x_t[nt][k][:],
                            in_=xT_r[:, k, nt * 512:(nt + 1) * 512],
                        )
                qT_f = [qT_sb, kT_sb]  # views picked below
                for u in range(4):
                    # q features (mf=u) and k features (mf=4+u)
                    for mf in (u, 4 + u):
                        dst = qT_sb if mf < 4 else kT_sb
                        pss = [p1ps.tile([128, 512], F32, name=f"qk{mf}_{nt}", tag="qk") for nt in range(NT)]
                        for k in range(KF):
                            for nt in range(NT):
                                nc.tensor.matmul(
                                    out=pss[nt][:],
                                    lhsT=wk_t[k][:, mf * 128:(mf + 1) * 128],
                                    rhs=x_t[nt][k][:],
                                    start=(k == 0),
                                    stop=(k == KF - 1),
                                    skip_group_check=True,
                                )
                        for nt in range(NT):
                            nc.vector.tensor_scalar(
                                out=dst[:, mf % 4, nt * 512:(nt + 1) * 512],
                                in0=pss[nt][:],
                                scalar1=bqk_sb[:, mf:mf + 1],
                                scalar2=None,
                                op0=mybir.AluOpType.add,
                            )
                    # v for this pair's 4 token m-tiles
                    for j in range(4):
                        mt = u * 4 + j
                        nt, jj = mt // 4, mt % 4
                        ps = p1vps.tile([128, 512], F32, tag="v")
                        for k in range(KF):
                            nc.tensor.matmul(
                                out=ps[:],
                                lhsT=x_t[nt][k][:, jj * 128:(jj + 1) * 128],
                                rhs=wk_t[k][:, 2 * CL:3 * CL],
                                start=(k == 0),
                                stop=False,
                            )
                        nc.tensor.matmul(
                            out=ps[:],
                            lhsT=ones_sb[0:1, :],
                            rhs=bv_sb[:],
                            start=False,
                            stop=True,
                        )
                        va = v_aug[mt][:].rearrange("p (h e) -> p h e", e=65)
                        nc.vector.tensor_copy(
                            va[:, :, 0:64],
                            ps[:].rearrange("p (h e) -> p h e", e=64),
                        )
                        nc.sync.dma_start(
                            out=va[:, :, 64:65],
                            in_=ones[:, 0:HL].unsqueeze(2),
                        )

            # ------------- phase 2+3: attention fused with proj -------------
            # q-tiles processed in fused groups {0,1} and {2,3}: one
            # v_aug/kT LDWEIGHTS serves every q-tile in the group (a full-row
            # LDWEIGHTS cannot overlap an in-flight matmul, so stationary
            # reuse is the only way to hide the ~290ns reload).
            with (
                tc.tile_pool(name="p23w", bufs=1) as p23w,
                tc.tile_pool(name="p2s", bufs=2, space="PSUM") as p2s,
                tc.tile_pool(name="p2y", bufs=1, space="PSUM") as p2y,
                tc.tile_pool(name="p2p", bufs=3) as p2p,
                tc.tile_pool(name="p2n", bufs=3) as p2n,
                tc.tile_pool(name="p2yt", bufs=2) as p2yt,
                tc.tile_pool(name="p3o", bufs=3) as p3o,
            ):
                wp_sb = p23w.tile([128, 4, C], F16, tag="wp")  # 8 KB/part
                nc.sync.dma_start(
                    out=wp_sb[:],
                    in_=wp.rearrange("(ko ki) f -> ki ko f", ki=128),
                )
                for qjs in ((0, 1), (2, 3)):
                    yT_cur = {qj: p2yt.tile([128, 4, 512], F16, name=f"yt{qj}", tag="yt")
                              for qj in qjs}
                    pending = []

                    def _norm(u, qj, yw, zi):
                        # zb2[0:64]=1/Z_A, zb2[64:128]=1/Z_B; borrows an s slot
                        zb2 = p2s.tile([128, 512], F32, name=f"zb{u}_{qj}", tag="s")
                        nc.tensor.matmul(
                            out=zb2[:],
                            lhsT=selAB_sb[64:66, :],
                            rhs=zi[64:66, :],
                            start=True, stop=True,
                            skip_group_check=True,
                        )
                        nc.vector.tensor_tensor(
                            out=yT_cur[qj][:, u, :],
                            in0=yw[:],
                            in1=zb2[:],
                            op=mybir.AluOpType.mult,
                        )

                    def _finish_pair(u, qj, yzA, yzB):
                        # gather y halves; B shifts partitions 0:64 -> 64:128
                        # via SBUF->SBUF DMA (engines cannot cross partitions)
                        yw = p2n.tile([128, 512], F32, tag="yw")
                        ywB = p2n.tile([64, 512], F32, tag="ywB")
                        zp = p2n.tile([66, 512], F32, tag="zp")
                        zq = p2n.tile([65, 512], F32, tag="zq")
                        nc.vector.tensor_copy(yw[0:64, :], yzA[0:64, :])
                        nc.vector.tensor_copy(ywB[:], yzB[0:64, :])
                        nc.vector.tensor_copy(zp[64:65, :], yzA[64:65, :])
                        nc.vector.tensor_copy(zq[64:65, :], yzB[64:65, :])
                        nc.sync.dma_start(out=yw[64:128, :], in_=ywB[:])
                        nc.sync.dma_start(out=zp[65:66, :], in_=zq[64:65, :])
                        # 1/Z for both heads in one ACT pass each
                        zl = p2n.tile([66, 512], F32, tag="zl")
                        nc.scalar.activation(out=zl[64:66, :], in_=zp[64:66, :], func=LOG)
                        zi = p2n.tile([66, 512], F16, tag="zi")
                        nc.scalar.activation(out=zi[64:66, :], in_=zl[64:66, :], func=EXP, scale=-1.0)
                        pending.append(lambda u=u, qj=qj, yw=yw, zi=zi: _norm(u, qj, yw, zi))

                    for u in range(4):          # head pair (2u, 2u+1), fb=u
                        hA, hB = 2 * u, 2 * u + 1
                        yz = {}
                        for qj in qjs:
                            yz[qj] = (
                                p2y.tile([65, 512], F32, name=f"yzA{u}_{qj}", tag=f"yzA{qj % 2}", bufs=1),
                                p2y.tile([65, 512], F32, name=f"yzB{u}_{qj}", tag=f"yzB{qj % 2}", bufs=1),
                            )
                        max_ki = 4 * qjs[-1] + 4
                        for ki in range(max_ki):
                            valid = [qj for qj in qjs if ki < 4 * qj + 4]
                            ps_ = {}
                            for qj in valid:
                                s = p2s.tile([128, 1024], F32, tag="s")
                                p = p2p.tile([128, 1024], F16, tag="p")
                                nc.tensor.matmul(
                                    out=s[:, 0:512],
                                    lhsT=kT_sb[0:64, u, ki * 128:(ki + 1) * 128],
                                    rhs=qT_sb[0:64, u, qj * 512:(qj + 1) * 512],
                                    start=True, stop=True,
                                    skip_group_check=True,
                                )
                                nc.tensor.matmul(
                                    out=s[:, 512:1024],
                                    lhsT=kT_sb[64:128, u, ki * 128:(ki + 1) * 128],
                                    rhs=qT_sb[64:128, u, qj * 512:(qj + 1) * 512],
                                    start=True, stop=True,
                                    skip_group_check=True,
                                )
                                nc.scalar.activation(out=p[:], in_=s[:], func=EXP)
                                i = ki - 4 * qj
                                if i >= 0:
                                    for half in range(2):
                                        ph = p[:, half * 512:(half + 1) * 512]
                                        nc.gpsimd.affine_select(
                                            out=ph, in_=ph,
                                            compare_op=mybir.AluOpType.is_ge,
                                            fill=0.0, base=-128 * i,
                                            pattern=[[1, 512]],
                                            channel_multiplier=-1,
                                        )
                                ps_[qj] = p
                            while pending:
                                pending.pop(0)()
                            # AVs: one LDWEIGHTS per head serves all q-tiles
                            for half, h in ((0, hA), (1, hB)):
                                for qj in valid:
                                    nc.tensor.matmul(
                                        out=yz[qj][half][0:65, :],
                                        lhsT=v_aug[ki][:, h * 65:(h + 1) * 65],
                                        rhs=ps_[qj][:, half * 512:(half + 1) * 512],
                                        start=(ki == 0),
                                        stop=(ki == 4 * qj + 3),
                                        skip_group_check=True,
                                    )
                            for qj in valid:
                                if ki == 4 * qj + 3:
                                    _finish_pair(u, qj, yz[qj][0], yz[qj][1])
                    while pending:
                        pending.pop(0)()
                    # proj for both q-tiles of the group
                    for qj in qjs:
                        for j4 in range(4):
                            mt = qj * 4 + j4
                            for no in range(2):
                                ps = p2s.tile([128, 512], F32, name=f"o{mt}_{no}", tag="s")
                                for kf in range(4):
                                    nc.tensor.matmul(
                                        out=ps[:],
                                        lhsT=yT_cur[qj][:, kf, j4 * 128:(j4 + 1) * 128],
                                        rhs=wp_sb[:, kf, no * 512:(no + 1) * 512],
                                        start=(kf == 0),
                                        stop=(kf == 3),
                                    )
                                os_ = p3o.tile([128, 512], F32, tag="os")
                                nc.vector.tensor_copy(os_[:], ps[:])
                                nc.sync.dma_start(
                                    out=o[mt * 128:(mt + 1) * 128, no * 512:(no + 1) * 512],
                                    in_=os_[:],
                                )

    if split_for_hw:
        _split_waits(nc)
    return nc


_NC = None


def _get_nc():
    global _NC
    if _NC is None:
        _NC = build()
    return _NC


def _host_prep(x, w_qkv, b_qkv, w_proj):
    """Build the 8 per-core input maps (fp16 for all GEMM operands)."""
    scale = 1.0 / np.sqrt(np.float32(D))
    ones = np.ones((128, 128), dtype=np.float16)
    selAB = np.zeros((128, 128), dtype=np.float16)
    selAB[64, 0:64] = 1.0    # Z_A (partition 64) -> zb2 rows 0:64
    selAB[65, 64:128] = 1.0  # Z_B (partition 65) -> zb2 rows 64:128
    in_maps = []
    for c in range(N_CORES):
        b, hg = c // 2, c % 2
        sl = slice(hg * CL, (hg + 1) * CL)
        wq = w_qkv[:, 0 * C:1 * C][:, sl] * scale
        wk_ = w_qkv[:, 1 * C:2 * C][:, sl]
        wv = w_qkv[:, 2 * C:3 * C][:, sl]
        bq = b_qkv[0 * C:1 * C][sl] * scale
        bk = b_qkv[1 * C:2 * C][sl]
        bvv = b_qkv[2 * C:3 * C][sl]
        in_maps.append({
            "xT": np.ascontiguousarray(x[b].T).astype(np.float16),
            "wk": np.ascontiguousarray(
                np.concatenate([wq, wk_, wv], axis=1)
            ).astype(np.float16),
            "bqk": np.ascontiguousarray(
                np.concatenate([bq, bk]).reshape(8, 128).T
            ).astype(np.float32),
            "bv": bvv.reshape(1, CL).astype(np.float16),
            "wp": np.ascontiguousarray(w_proj[sl, :]).astype(np.float16),
            "ones": ones,
            "selAB": selAB,
        })
    return in_maps


def kernel(x, w_qkv, b_qkv, w_proj, b_proj, _trace=False):
    x = np.asarray(x, dtype=np.float32)
    w_qkv = np.asarray(w_qkv, dtype=np.float32)
    b_qkv = np.asarray(b_qkv, dtype=np.float32)
    w_proj = np.asarray(w_proj, dtype=np.float32)
    b_proj = np.asarray(b_proj, dtype=np.float32)

    nc = _get_nc()
    in_maps = _host_prep(x, w_qkv, b_qkv, w_proj)
    res = run_bass_kernel_spmd(nc, in_maps, list(range(N_CORES)), trace=_trace)
    out = np.empty((B, T, C), dtype=np.float32)
    for b in range(B):
        out[b] = res.results[2 * b]["o"] + res.results[2 * b + 1]["o"]
    out += b_proj[None, None, :]
    if _trace:
        kernel.last_exec_time_ns = res.exec_time_ns
        kernel.last_results = res
    return out


# revision 20
# speedup vs baseline: 1.1921x; 1.1921x over previous
"""Causal self-attention (B=4, T=2048, C=1024, H=16, D=64) on 8 NeuronCores.

Sharding: core c -> batch b = c//2 (data parallel), head-group hg = c%2
(tensor parallel: heads hg*8..hg*8+7; column-parallel qkv, row-parallel
proj). Each core computes a partial [T, C] projection output; the host
sums the two partials per batch (row-parallel all-reduce done host-side
during unshard) and adds b_proj.

All GEMMs run in float16 (1 cycle/row on the PE vs 2 passes for
fp32/fp32r; 11-bit mantissa keeps the end-to-end error ~1e-3).
Accumulation is always fp32 in PSUM.

Per-core device program (identical on all cores; only data differs):
  phase 1: qT,kT feature-major [128,4,T]; v token-major augmented with a
           ones column per head -> v_aug [T, 8*65]
  phase 2: per (q-tile qj of 512, head h): scores transposed
           sT[k_tok, q] = kT.T @ qT via K=64 matmuls; one Exp pass per
           [128,1024] PSUM group (softmax max-subtraction is skipped:
           scores are ~N(0,1) since 1/sqrt(D) is folded into w_q, so
           exp never overflows fp32 and softmax is shift-invariant;
           fp16 overflow in not-yet-masked upper-triangle entries is
           zeroed right after); causal masking via gpsimd affine_select
           on the diagonal blocks of p; AV accumulates yz[65, 512] with
           v_aug stationary -- row 64 is the softmax denominator Z;
           1/Z via ACT exp(-log(Z)); broadcast over partitions via a
           K=1 ones matmul; normalize on DVE; write yT feature-major.
  phase 3 (fused in the qj loop): out[512 tok slice, C] = yT.T @ wp.
"""

import numpy as np

import concourse.bass as bass
import concourse.mybir as mybir
from concourse.tile import TileContext
from concourse.bass_utils import run_bass_kernel_spmd

F32 = mybir.dt.float32
F16 = mybir.dt.float16

B, T, C = 4, 2048, 1024
H, D = 16, 64
HL = 8            # heads per core
CL = HL * D       # 512 local feature width

N_CORES = 8
NT = T // 512     # 4 q-tiles / token n-tiles of 512
MT = T // 128     # 16 token m-tiles
KF = C // 128     # 8 contraction tiles over C


def _split_waits(nc, cap=1):
    """walrus codegen limits sync-wait commands per ISA instruction.
    Hoist excess waits onto preceding same-engine nops."""
    n = 0
    for f in nc.m.functions:
        for blk in f.blocks:
            out = []
            for inst in blk.instructions:
                si = getattr(inst, "sync_info", None)
                if si is not None and si.on_wait and len(si.on_wait) > cap:
                    extra = list(si.on_wait[:-cap])
                    keep = list(si.on_wait[-cap:])
                    for w in extra:
                        nop = mybir.InstNoOp(name=f"wsplit-{n}", ins=[], outs=[])
                        nop.engine = inst.engine
                        nop.sync_info = mybir.SyncInfo(on_wait=[w], on_update=[])
                        out.append(nop)
                        n += 1
                    inst.sync_info = mybir.SyncInfo(
                        on_wait=keep, on_update=list(si.on_update)
                    )
                out.append(inst)
            blk.instructions = out
    return n


def build(split_for_hw=True):
    nc = bass.Bass()
    xT = nc.declare_dram_parameter("xT", [C, T], F16, isOutput=False)
    wk = nc.declare_dram_parameter("wk", [C, 3 * CL], F16, isOutput=False)
    bqk = nc.declare_dram_parameter("bqk", [128, 8], F32, isOutput=False)
    bv2 = nc.declare_dram_parameter("bv2", [128, 4], F32, isOutput=False)
    idn = nc.declare_dram_parameter("idn", [128, 128], F16, isOutput=False)
    wp = nc.declare_dram_parameter("wp", [CL, C], F16, isOutput=False)
    ones = nc.declare_dram_parameter("ones", [128, 128], F16, isOutput=False)
    selAB = nc.declare_dram_parameter("selAB", [128, 128], F16, isOutput=False)
    o = nc.declare_dram_parameter("o", [T, C], F32, isOutput=True)

    EXP = mybir.ActivationFunctionType.Exp
    LOG = mybir.ActivationFunctionType.Ln

    with TileContext(nc) as tc:
        with (
            tc.tile_pool(name="const", bufs=1) as cpool,
            tc.tile_pool(name="big", bufs=1) as big,
        ):
            # constants
            ones_sb = cpool.tile([128, 128], F16, tag="ones")
            bqk_sb = cpool.tile([128, 8], F32, tag="bqk")
            bv2_sb = cpool.tile([128, 4], F32, tag="bv2")
            identity_sb = cpool.tile([128, 128], F16, tag="idn")
            selAB_sb = cpool.tile([128, 128], F16, tag="selAB")
            nc.sync.dma_start(out=selAB_sb[:], in_=selAB[:])
            nc.sync.dma_start(out=ones_sb[:], in_=ones[:])
            nc.sync.dma_start(out=bqk_sb[:], in_=bqk[:])
            nc.sync.dma_start(out=bv2_sb[:], in_=bv2[:])
            nc.sync.dma_start(out=identity_sb[:], in_=idn[:])

            # persistent activations
            qT_sb = big.tile([128, NT, T], F16, tag="qT")    # 16 KB/part
            kT_sb = big.tile([128, NT, T], F16, tag="kT")    # 16 KB/part
            v_aug = [
                big.tile([128, HL * 65], F16, name=f"vaug{mt}", tag=f"vaug{mt}")
                for mt in range(MT)
            ]

            # ---------------- phase 1: qkv ----------------
            # mf-pair-outer order: feature block fb=u (q cols mf=u, k cols
            # mf=4+u) completes before pair u's attention needs it, so
            # phase 2 overlaps the back of phase 1. x stays fully resident
            # (fp16) as per-(nt,k) tiles for fine-grained DMA deps.
            with (
                tc.tile_pool(name="p1w", bufs=1) as p1w,
                tc.tile_pool(name="p1x", bufs=1) as p1x,
                tc.tile_pool(name="p1ps", bufs=4, space="PSUM") as p1ps,
                tc.tile_pool(name="p1vps", bufs=2, space="PSUM") as p1vps,
                tc.tile_pool(name="p1vs", bufs=2) as p1vs,
            ):
                wk_t = [
                    p1w.tile([128, 3 * CL], F16, name=f"wk{k}", tag=f"wk{k}")
                    for k in range(KF)
                ]
                wk_r = wk.rearrange("(ko ki) f -> ki ko f", ki=128)
                x_t = [
                    [
                        p1x.tile([128, 512], F16, name=f"x{nt}_{k}", tag=f"x{nt}_{k}")
                        for k in range(KF)
                    ]
                    for nt in range(NT)
                ]
                xT_r = xT.rearrange("(ko ki) t -> ki ko t", ki=128)
                for k in range(KF):
                    nc.sync.dma_start(out=wk_t[k][:], in_=wk_r[:, k, :])
                    for nt in range(NT):
                        nc.sync.dma_start(
                            out=x_t[nt][k][:],
                            in_=xT_r[:, k, nt * 512:(nt + 1) * 512],
                        )
                qT_f = [qT_sb, kT_sb]  # views picked below
                for u in range(4):
                    # q features (mf=u) and k features (mf=4+u)
                    for mf in (u, 4 + u):
                        dst = qT_sb if mf < 4 else kT_sb
                        pss = [p1ps.tile([128, 512], F32, name=f"qk{mf}_{nt}", tag="qk") for nt in range(NT)]
                        for k in range(KF):
                            for nt in range(NT):
                                nc.tensor.matmul(
                                    out=pss[nt][:],
                                    lhsT=wk_t[k][:, mf * 128:(mf + 1) * 128],
                                    rhs=x_t[nt][k][:],
                                    start=(k == 0),
                                    stop=(k == KF - 1),
                                    skip_group_check=True,
                                )
                        for nt in range(NT):
                            nc.vector.tensor_scalar(
                                out=dst[:, mf % 4, nt * 512:(nt + 1) * 512],
                                in0=pss[nt][:],
                                scalar1=bqk_sb[:, mf:mf + 1],
                                scalar2=None,
                                op0=mybir.AluOpType.add,
                            )
                    # v for this pair's 4 token m-tiles: compute vT
                    # feature-major with wv stationary (LDWEIGHTS reused over
                    # token tiles), then PE-transpose to token-major (identity
                    # stationary, loaded once)
                    nt = u
                    vts = p1vs.tile([128, 4, 512], F16, tag="vts")
                    for vf in range(4):
                        vps = p1vps.tile([128, 512], F32, tag="vt", bufs=2)
                        for k in range(KF):
                            nc.tensor.matmul(
                                out=vps[:],
                                lhsT=wk_t[k][:, 2 * CL + vf * 128:2 * CL + (vf + 1) * 128],
                                rhs=x_t[nt][k][:],
                                start=(k == 0),
                                stop=(k == KF - 1),
                                skip_group_check=True,
                            )
                        nc.vector.tensor_scalar(
                            out=vts[:, vf, :],
                            in0=vps[:],
                            scalar1=bv2_sb[:, vf:vf + 1],
                            scalar2=None,
                            op0=mybir.AluOpType.add,
                        )
                    # transpose 16 [128,128] blocks: vts[vf, tok] -> v_aug
                    for j in range(4):
                        mt = nt * 4 + j
                        tp = p1vps.tile([128, 4, 128], F16, tag="tp", bufs=2)
                        for vf in range(4):
                            nc.tensor.transpose(
                                tp[:, vf, :],
                                vts[:, vf, j * 128:(j + 1) * 128],
                                identity_sb[:],
                            )
                        va = v_aug[mt][:].rearrange("p (h e) -> p h e", e=65)
                        nc.vector.tensor_copy(
                            va[:, :, 0:64],
                            tp[:].rearrange("p g e -> p (g e)").rearrange("p (h e) -> p h e", e=64),
                        )
                        nc.sync.dma_start(
                            out=va[:, :, 64:65],
                            in_=ones[:, 0:HL].unsqueeze(2),
                        )

            # ------------- phase 2+3: attention fused with proj -------------
            with (
                tc.tile_pool(name="p23w", bufs=1) as p23w,
                tc.tile_pool(name="p2s", bufs=2, space="PSUM") as p2s,
                tc.tile_pool(name="p2y", bufs=1, space="PSUM") as p2y,
                tc.tile_pool(name="pzp", bufs=2, space="PSUM") as pzp,
                tc.tile_pool(name="p2p", bufs=3) as p2p,
                tc.tile_pool(name="p2n", bufs=3) as p2n,
                tc.tile_pool(name="p2yt", bufs=2) as p2yt,
                tc.tile_pool(name="p3o", bufs=3) as p3o,
            ):
                wp_sb = p23w.tile([128, 4, C], F16, tag="wp")  # 8 KB/part
                nc.sync.dma_start(
                    out=wp_sb[:],
                    in_=wp.rearrange("(ko ki) f -> ki ko f", ki=128),
                )
                for qj in range(NT):
                    # yT for this q-tile: [feat 128, feat-block 4, tok 512]
                    yT_cur = p2yt.tile([128, 4, 512], F16, tag="yt")
                    nki = 4 * qj + 4
                    pending_norm = None

                    def _norm(u, yw, zi):
                        # zb2[0:64] = 1/Z_A, zb2[64:128] = 1/Z_B (K=2 matmul)
                        zb2 = pzp.tile([128, 512], F32, tag="zbproj")
                        nc.tensor.matmul(
                            out=zb2[:],
                            lhsT=selAB_sb[64:66, :],
                            rhs=zi[64:66, :],
                            start=True, stop=True,
                            skip_group_check=True,
                        )
                        nc.vector.tensor_tensor(
                            out=yT_cur[:, u, :],
                            in0=yw[:],
                            in1=zb2[:],
                            op=mybir.AluOpType.mult,
                        )

                    for u in range(4):          # head pair (2u, 2u+1), fb=u
                        hA, hB = 2 * u, 2 * u + 1
                        yzA = p2y.tile([128, 512], F32, tag="yzA", bufs=1)
                        yzB = p2y.tile([128, 512], F32, tag="yzB", bufs=1)
                        for ki in range(nki):
                            i = ki - 4 * qj
                            s = p2s.tile([128, 1024], F32, tag="s")
                            p = p2p.tile([128, 1024], F16, tag="p")
                            # two K=64 matmuls packed in one array pass
                            # (rows 0-63 for head A, 64-127 for head B)
                            nc.tensor.matmul(
                                out=s[:, 0:512],
                                lhsT=kT_sb[0:64, u, ki * 128:(ki + 1) * 128],
                                rhs=qT_sb[0:64, u, qj * 512:(qj + 1) * 512],
                                start=True, stop=True,
                                skip_group_check=True,
                            )
                            nc.tensor.matmul(
                                out=s[:, 512:1024],
                                lhsT=kT_sb[64:128, u, ki * 128:(ki + 1) * 128],
                                rhs=qT_sb[64:128, u, qj * 512:(qj + 1) * 512],
                                start=True, stop=True,
                                skip_group_check=True,
                            )
                            nc.scalar.activation(out=p[:], in_=s[:], func=EXP)
                            for half, (yzt, h) in enumerate(((yzA, hA), (yzB, hB))):
                                ph = p[:, half * 512:(half + 1) * 512]
                                if i >= 0:
                                    # zero the non-causal (q - k < 128*i) part
                                    nc.gpsimd.affine_select(
                                        out=ph, in_=ph,
                                        compare_op=mybir.AluOpType.is_ge,
                                        fill=0.0, base=-128 * i,
                                        pattern=[[1, 512]],
                                        channel_multiplier=-1,
                                    )
                                nc.tensor.matmul(
                                    out=yzt[0:65, :],
                                    lhsT=v_aug[ki][:, h * 65:(h + 1) * 65],
                                    rhs=ph,
                                    start=(ki == 0),
                                    stop=(ki == nki - 1),
                                    skip_group_check=True,
                                )
                            if ki == 0 and pending_norm is not None:
                                # previous pair's zb2+mult, emitted here so the
                                # PE never waits on the ACT 1/Z chain
                                pending_norm()
                                pending_norm = None
                        # gather y halves; B shifts partitions 0:64 ->
                        # 64:128 via SBUF->SBUF DMA (engines cannot cross
                        # partitions; DMA can)
                        yw = p2n.tile([128, 512], F32, tag="yw")
                        ywB = p2n.tile([64, 512], F32, tag="ywB")
                        zp = p2n.tile([66, 512], F32, tag="zp")
                        zq = p2n.tile([65, 512], F32, tag="zq")
                        nc.vector.tensor_copy(yw[0:64, :], yzA[0:64, :])
                        nc.vector.tensor_copy(ywB[:], yzB[0:64, :])
                        nc.vector.tensor_copy(zp[64:65, :], yzA[64:65, :])
                        nc.vector.tensor_copy(zq[64:65, :], yzB[64:65, :])
                        nc.sync.dma_start(out=yw[64:128, :], in_=ywB[:])
                        nc.sync.dma_start(out=zp[65:66, :], in_=zq[64:65, :])
                        # 1/Z for both heads in one ACT pass each
                        zl = p2n.tile([66, 512], F32, tag="zl")
                        nc.scalar.activation(out=zl[64:66, :], in_=zp[64:66, :], func=LOG)
                        zi = p2n.tile([66, 512], F16, tag="zi")
                        nc.scalar.activation(out=zi[64:66, :], in_=zl[64:66, :], func=EXP, scale=-1.0)
                        pending_norm = (lambda u=u, yw=yw, zi=zi: _norm(u, yw, zi))
                    if pending_norm is not None:
                        pending_norm()
                        pending_norm = None
                    # proj for this q-tile's 4 token m-tiles
                    for j4 in range(4):
                        mt = qj * 4 + j4
                        for no in range(2):
                            ps = pzp.tile([128, 512], F32, tag="zbproj")
                            for kf in range(4):
                                nc.tensor.matmul(
                                    out=ps[:],
                                    lhsT=yT_cur[:, kf, j4 * 128:(j4 + 1) * 128],
                                    rhs=wp_sb[:, kf, no * 512:(no + 1) * 512],
                                    start=(kf == 0),
                                    stop=(kf == 3),
                                )
                            os_ = p3o.tile([128, 512], F32, tag="os")
                            nc.vector.tensor_copy(os_[:], ps[:])
                            nc.sync.dma_start(
                                out=o[mt * 128:(mt + 1) * 128, no * 512:(no + 1) * 512],
                                in_=os_[:],
                            )

    if split_for_hw:
        _split_waits(nc)
    return nc


_NC = None


def _get_nc():
    global _NC
    if _NC is None:
        _NC = build()
    return _NC


def _host_prep(x, w_qkv, b_qkv, w_proj):
    """Build the 8 per-core input maps (fp16 for all GEMM operands)."""
    scale = 1.0 / np.sqrt(np.float32(D))
    ones = np.ones((128, 128), dtype=np.float16)
    selAB = np.zeros((128, 128), dtype=np.float16)
    selAB[64, 0:64] = 1.0    # Z_A (partition 64) -> zb2 rows 0:64
    selAB[65, 64:128] = 1.0  # Z_B (partition 65) -> zb2 rows 64:128
    in_maps = []
    for c in range(N_CORES):
        b, hg = c // 2, c % 2
        sl = slice(hg * CL, (hg + 1) * CL)
        wq = w_qkv[:, 0 * C:1 * C][:, sl] * scale
        wk_ = w_qkv[:, 1 * C:2 * C][:, sl]
        wv = w_qkv[:, 2 * C:3 * C][:, sl]
        bq = b_qkv[0 * C:1 * C][sl] * scale
        bk = b_qkv[1 * C:2 * C][sl]
        bvv = b_qkv[2 * C:3 * C][sl]
        in_maps.append({
            "xT": np.ascontiguousarray(x[b].T).astype(np.float16),
            "wk": np.ascontiguousarray(
                np.concatenate([wq, wk_, wv], axis=1)
            ).astype(np.float16),
            "bqk": np.ascontiguousarray(
                np.concatenate([bq, bk]).reshape(8, 128).T
            ).astype(np.float32),
            "bv2": np.ascontiguousarray(bvv.reshape(4, 128).T).astype(np.float32),
            "idn": np.eye(128, dtype=np.float16),
            "wp": np.ascontiguousarray(w_proj[sl, :]).astype(np.float16),
            "ones": ones,
            "selAB": selAB,
        })
    return in_maps


def kernel(x, w_qkv, b_qkv, w_proj, b_proj, _trace=False):
    x = np.asarray(x, dtype=np.float32)
    w_qkv = np.asarray(w_qkv, dtype=np.float32)
    b_qkv = np.asarray(b_qkv, dtype=np.float32)
    w_proj = np.asarray(w_proj, dtype=np.float32)
    b_proj = np.asarray(b_proj, dtype=np.float32)

    nc = _get_nc()
    in_maps = _host_prep(x, w_qkv, b_qkv, w_proj)
    res = run_bass_kernel_spmd(nc, in_maps, list(range(N_CORES)), trace=_trace)
    out = np.empty((B, T, C), dtype=np.float32)
    for b in range(B):
        out[b] = res.results[2 * b]["o"] + res.results[2 * b + 1]["o"]
    out += b_proj[None, None, :]
    if _trace:
        kernel.last_exec_time_ns = res.exec_time_ns
        kernel.last_results = res
    return out
